# revision 49
# baseline (speedup 1.0000x reference)
"""HeatmapMSELoss Trainium2 kernel (fp8 stream + PE Gram-matmul squares).

Computes mean((heatmaps_pred - heatmaps_gt)^2) where heatmaps_gt is an
isotropic 2D gaussian (sigma=1, peak 1) rendered at the projection of each
3D joint into each view.

Key identity: the gaussian separates, gt[h,w] = gy[h] * gx[w], so

  sum_hw (pred - gt)^2 = sum_hw pred^2 - 2 * gy^T (pred @ gx) + (sum gy^2)(sum gx^2)

The gt tensor is never materialized. pred is pre-transposed on host to
h-major [H, S, W] and cast to fp8e4m3 (biases the final scalar by ~7e-4
relative, harness tolerance 2e-2), quartering HBM traffic vs f32 while
every DMA descriptor stays >= 512 contiguous bytes.

sum(pred^2) rides the (otherwise idle) PE systolic array: per group of
slices, PSUM-accumulated Gram matmuls G = sum_s pred_s^T pred_s, whose
trace is sum of squares. The diagonal is extracted once per group by one
DVE multiply against a preloaded identity mask and one reduce. The ACT
engine squares a minority share directly (activation Square + accum),
sized larger in the early groups while the PE array is still in its slow
p-state; DVE squares a few head slices it would otherwise idle through.
This makes the kernel memory-bound on the fp8 stream.

The cross term uses one per-slice matmul (pred_s^T @ gy_s) into a
persistent PSUM tile; the multiply by gx and the reduction run as two
grouped DVE ops.

Sharding: data-parallel over batch, 4 batches per core across 8 cores;
the tiny per-group column partials are combined on host in float64.
"""

import numpy as np
import ml_dtypes

import concourse.bacc as bacc
import concourse.bass as bass
import concourse.tile as tile
from concourse import mybir
from concourse.bass_utils import run_bass_kernel_spmd

B, V, J, H, W = 32, 4, 17, 128, 128
N_CORES = 8
B_LOC = B // N_CORES          # 4 batches per core
SLICES = B_LOC * V * J        # 272 slices per core

_CACHE = {}

# Per group: (pe_slices, act_slices, dve_slices). PE share is small in the
# first groups (array still ramping its p-state) and in the last (short
# tail after the final DMA lands).
GROUPS = [
    (12, 4, 4),
    (31, 13, 3),
    (33, 11, 3),
    (33, 11, 3),
    (33, 11, 3),
    (34, 11, 3),
    (4, 2, 2),
    (8, 0, 0),
]
assert sum(p + a + d for p, a, d in GROUPS) == SLICES
NG = len(GROUPS)
MAXSZ = max(p + a + d for p, a, d in GROUPS)
NGRP = NG - 3  # groups 0..NGRP-1 feed psA; the rest feed psB
N_WARM = 27    # dummy PE matmuls to ramp the array's p-state before work

# gx is scaled by -2 on host, so the cross-term prods add directly into the
# same sign as the squares and every partial column is a plain addend.
# outcols: [G diag | merged G2 diag + prodB | ACT (NG) | DVE sq (NG) | prodA]
NC = 2 * NG + 3


def _build_nc():
    nc = bacc.Bacc()
    f32 = mybir.dt.float32
    bf16 = mybir.dt.bfloat16
    fp8 = mybir.dt.float8e4

    pred8 = nc.declare_dram_parameter("pred8", [H, SLICES, W], fp8, isOutput=False)
    gy8 = nc.declare_dram_parameter("gy8", [H, SLICES], fp8, isOutput=False)
    gx = nc.declare_dram_parameter("gx", [W, SLICES], bf16, isOutput=False)
    ident = nc.declare_dram_parameter("ident", [128, 128], bf16, isOutput=False)
    partials = nc.declare_dram_parameter("partials", [128, NC], f32, isOutput=True)

    n_main = sum(sum(g) for g in GROUPS[:NGRP])
    n_last = SLICES - n_main

    with tile.TileContext(nc) as tc:
        with (
            tc.tile_pool(name="consts", bufs=1) as consts,
            tc.tile_pool(name="l8", bufs=6) as l8pool,
            tc.tile_pool(name="work", bufs=1) as work,
            tc.tile_pool(name="gpsum", bufs=1, space="PSUM") as gpsum,
            tc.tile_pool(name="mpsum", bufs=1, space="PSUM") as mpsum,
            tc.tile_pool(name="outs", bufs=1) as outs,
        ):
            # warm-up ACT so the Square table-set load overlaps the DMA ramp
            warm = consts.tile([128, 1], f32)
            nc.vector.memset(warm[:], 0.0)
            wsq = consts.tile([128, 1], f32)
            nc.scalar.activation(
                out=wsq[:], in_=warm[:], func=mybir.ActivationFunctionType.Square
            )
            # warm up the PE array: dummy matmuls ramp its p-state to full
            # clock while the first pred chunk is still in flight
            wmat = consts.tile([128, 128], fp8)
            nc.vector.memset(wmat[:], 0.0)
            Gw = gpsum.tile([128, 128], f32, tag="Gw")
            for i in range(N_WARM):
                nc.tensor.matmul(
                    Gw[:], wmat[:], wmat[:], start=True, stop=True
                )

            gy8_t = consts.tile([H, SLICES], fp8)
            gx_t = consts.tile([W, SLICES], bf16)
            ident_t = consts.tile([128, 128], bf16)
            actout = consts.tile([128, max(a for _, a, _ in GROUPS) * W], bf16)
            sq16 = consts.tile([128, max(d for _, _, d in GROUPS) * W], bf16)
            outcols = outs.tile([128, NC], f32)

            psA = mpsum.tile([128, n_main], f32, tag="psA")
            psB = mpsum.tile([128, n_last], f32, tag="psB")

            # global Gram accumulator for groups 0..NG-3 (diagonal read out
            # right after the last of those chains, off the tail); the last
            # two groups' PE shares go to a small separate accumulator so
            # the tail chain is short
            G = gpsum.tile([128, 128], f32, tag="G")
            G2 = gpsum.tile([128, 128], f32, tag="G2")
            n_pe_main = sum(p for p, _, _ in GROUPS[:-2])
            n_pe_tail = sum(p for p, _, _ in GROUPS[-2:])
            pe_done = 0
            pe2_done = 0

            g0 = 0
            mm0 = None
            for r, (pk, ak, dk) in enumerate(GROUPS):
                sz = pk + ak + dk
                t8 = l8pool.tile([128, MAXSZ * W], fp8, tag="l8")
                nc.sync.dma_start(
                    out=t8[:, : sz * W],
                    in_=pred8[:, g0 : g0 + sz, :].rearrange("h s w -> h (s w)"),
                )
                if r == 1:
                    # small const loads slot in after the first two pred
                    # DMAs; their consumers (PE cross-term matmuls, grouped
                    # prods, per-group diag extraction) all have slack
                    nc.sync.dma_start(out=gy8_t[:], in_=gy8[:, :])
                    nc.sync.dma_start(out=gx_t[:], in_=gx[:, :])
                    nc.sync.dma_start(out=ident_t[:], in_=ident[:, :])

                # PE: Gram accumulation over its share -> trace = sum sq
                if r < NG - 2:
                    for i in range(pk):
                        nc.tensor.matmul(
                            G[:],
                            t8[:, i * W : (i + 1) * W],
                            t8[:, i * W : (i + 1) * W],
                            start=(pe_done + i == 0),
                            stop=(pe_done + i == n_pe_main - 1),
                        )
                    pe_done += pk
                else:
                    for i in range(pk):
                        nc.tensor.matmul(
                            G2[:],
                            t8[:, i * W : (i + 1) * W],
                            t8[:, i * W : (i + 1) * W],
                            start=(pe2_done + i == 0),
                            stop=(pe2_done + i == n_pe_tail - 1),
                        )
                    pe2_done += pk

                # ACT share: activation Square with accumulator
                if ak:
                    nc.scalar.activation(
                        out=actout[:, : ak * W],
                        in_=t8[:, pk * W : (pk + ak) * W],
                        func=mybir.ActivationFunctionType.Square,
                        accum_out=outcols[:, 2 + r : 3 + r],
                    )
                else:
                    nc.vector.memset(outcols[:, 2 + r : 3 + r], 0.0)

                # DVE share: square (fp8 -> bf16), halving tree, reduce
                if dk:
                    n = dk * W
                    o = (pk + ak) * W
                    nc.vector.tensor_mul(
                        sq16[:, :n], t8[:, o : o + n], t8[:, o : o + n]
                    )
                    while n > 256:
                        h = n // 2
                        nc.vector.tensor_add(
                            sq16[:, :h], sq16[:, :h], sq16[:, h:n]
                        )
                        n = h
                    nc.vector.reduce_sum(
                        outcols[:, 2 + NG + r : 3 + NG + r], sq16[:, :n],
                        axis=mybir.AxisListType.X,
                    )
                else:
                    nc.vector.memset(outcols[:, 2 + NG + r : 3 + NG + r], 0.0)

                # cross term: m'_s = pred_s^T @ gy_s per slice -> psum column
                ps, col0 = (psA, g0) if r < NGRP else (psB, g0 - n_main)

                def emit_matmuls(ps=ps, col0=col0, t8=t8, sz=sz, g0=g0):
                    for i in range(sz):
                        nc.tensor.matmul(
                            ps[:, col0 + i : col0 + i + 1],
                            t8[:, i * W : (i + 1) * W],
                            gy8_t[:, g0 + i : g0 + i + 1],
                            start=True,
                            stop=True,
                        )

                if r == 0:
                    # gy is only loaded during group 1; defer group 0's
                    # cross-term matmuls past it in program order
                    mm0 = emit_matmuls
                else:
                    if r == 1:
                        mm0()
                    emit_matmuls()

                if r == NGRP:
                    # grouped prod over the main groups' m' columns; its
                    # matmul wait resolved long ago, no queue blocking
                    prodA = work.tile([128, n_main], f32, tag="prodA")
                    nc.vector.tensor_mul(prodA[:], psA[:], gx_t[:, :n_main])
                    nc.vector.reduce_sum(
                        outcols[:, 2 * NG + 2 : 2 * NG + 3], prodA[:],
                        axis=mybir.AxisListType.X,
                    )

                if r == NG - 2:
                    # global Gram diagonal: runs as soon as the main G
                    # chain stops, while the last group still streams
                    dbuf = work.tile([128, 128], f32, tag="dbuf")
                    nc.vector.tensor_mul(dbuf[:], G[:], ident_t[:])
                    nc.vector.reduce_sum(
                        outcols[:, 0:1], dbuf[:], axis=mybir.AxisListType.X
                    )

                g0 += sz

            # tail: G2 diagonal and last groups' prod share one buffer and
            # one reduce (gx carries the -2, so both are plain s1 addends)
            tbuf = work.tile([128, 128 + n_last], f32, tag="tbuf")
            nc.vector.tensor_mul(tbuf[:, :128], G2[:], ident_t[:])
            nc.vector.tensor_mul(tbuf[:, 128:], psB[:], gx_t[:, n_main:])
            nc.vector.reduce_sum(
                outcols[:, 1:2], tbuf[:], axis=mybir.AxisListType.X
            )

            nc.sync.dma_start(out=partials[:, :], in_=outcols[:])

    nc.finalize()
    return nc


def _gaussians(proj_mats_batch, joints_3d_gt_batch):
    """1D gaussians gy [B,V,J,H], gx [B,V,J,W] in float32 (reference math)."""
    joints = joints_3d_gt_batch.astype(np.float32)
    ones = np.ones(joints.shape[:-1] + (1,), dtype=np.float32)
    joints_h = np.concatenate([joints, ones], axis=-1)  # [B, J, 4]
    proj = np.einsum(
        "bvcd,bjd->bvjc", proj_mats_batch.astype(np.float32), joints_h
    ).astype(np.float32)  # [B, V, J, 3]
    joints_2d = proj[..., :2] / proj[..., 2:3]  # (x, y)
    xs = np.arange(W, dtype=np.float32)
    ys = np.arange(H, dtype=np.float32)
    dx2 = (xs - joints_2d[..., 0, None]) ** 2  # [B,V,J,W]
    dy2 = (ys - joints_2d[..., 1, None]) ** 2  # [B,V,J,H]
    gx = np.exp(-0.5 * dx2).astype(np.float32)
    gy = np.exp(-0.5 * dy2).astype(np.float32)
    return gy, gx


def kernel(heatmaps_pred, proj_mats_batch, joints_3d_gt_batch, joints_3d_valid_batch,
           _profile=None):
    heatmaps_pred = np.asarray(heatmaps_pred, dtype=np.float32)
    gy, gx = _gaussians(np.asarray(proj_mats_batch), np.asarray(joints_3d_gt_batch))

    # s3 = sum over slices of (sum_h gy^2) * (sum_w gx^2), exact in f64
    s3 = float(
        ((gy.astype(np.float64) ** 2).sum(-1) * (gx.astype(np.float64) ** 2).sum(-1)).sum()
    )

    if "nc" not in _CACHE:
        _CACHE["nc"] = _build_nc()
    nc = _CACHE["nc"]

    ident = np.eye(128, dtype=ml_dtypes.bfloat16)
    in_maps = []
    for c in range(N_CORES):
        bsl = slice(B_LOC * c, B_LOC * (c + 1))
        # slice order: (b_local, v, j) -> s ; pred h-major [H, SLICES, W]
        pred8 = np.ascontiguousarray(
            heatmaps_pred[bsl].reshape(SLICES, H, W).transpose(1, 0, 2)
            .astype(ml_dtypes.float8_e4m3)
        )
        gy8 = np.ascontiguousarray(
            gy[bsl].reshape(SLICES, H).T.astype(ml_dtypes.float8_e4m3)
        )
        gx_c = np.ascontiguousarray(
            (-2.0 * gx[bsl].reshape(SLICES, W).T).astype(ml_dtypes.bfloat16)
        )
        in_maps.append({"pred8": pred8, "gy8": gy8, "gx": gx_c, "ident": ident})

    res = run_bass_kernel_spmd(nc, in_maps, core_ids=list(range(N_CORES)))
    if _profile is not None:
        _profile["result"] = res
        _profile["in_maps"] = in_maps

    # every partials column is a plain addend: squares partials plus the
    # cross-term prods (gx already carries the -2)
    total = s3
    for c in range(N_CORES):
        total += res.results[c]["partials"].astype(np.float64).sum()
    return np.float32(total / (B * V * J * H * W))


# revision 50
# speedup vs baseline: 1.0387x; 1.0387x over previous
"""HeatmapMSELoss Trainium2 kernel (fp8 stream + PE Gram-matmul squares).

Computes mean((heatmaps_pred - heatmaps_gt)^2) where heatmaps_gt is an
isotropic 2D gaussian (sigma=1, peak 1) rendered at the projection of each
3D joint into each view.

Key identity: the gaussian separates, gt[h,w] = gy[h] * gx[w], so

  sum_hw (pred - gt)^2 = sum_hw pred^2 - 2 * gy^T (pred @ gx) + (sum gy^2)(sum gx^2)

The gt tensor is never materialized. pred is pre-transposed on host to
h-major [H, S, W] and cast to fp8e4m3 (biases the final scalar by ~7e-4
relative, harness tolerance 2e-2), quartering HBM traffic vs f32 while
every DMA descriptor stays >= 512 contiguous bytes.

sum(pred^2) rides the (otherwise idle) PE systolic array: per group of
slices, PSUM-accumulated Gram matmuls G = sum_s pred_s^T pred_s, whose
trace is sum of squares. The diagonal is extracted once per group by one
DVE multiply against a preloaded identity mask and one reduce. The ACT
engine squares a minority share directly (activation Square + accum),
sized larger in the early groups while the PE array is still in its slow
p-state; DVE squares a few head slices it would otherwise idle through.
This makes the kernel memory-bound on the fp8 stream.

The cross term uses one per-slice matmul (pred_s^T @ gy_s) into a
persistent PSUM tile; the multiply by gx and the reduction run as two
grouped DVE ops.

Sharding: data-parallel over batch, 4 batches per core across 8 cores;
the tiny per-group column partials are combined on host in float64.
"""

import numpy as np
import ml_dtypes

import concourse.bacc as bacc
import concourse.bass as bass
import concourse.tile as tile
from concourse import mybir
from concourse.bass_utils import run_bass_kernel_spmd

B, V, J, H, W = 32, 4, 17, 128, 128
N_CORES = 8
B_LOC = B // N_CORES          # 4 batches per core
SLICES = B_LOC * V * J        # 272 slices per core

_CACHE = {}

# Per group: (pe_slices, act_slices, dve_slices). PE share is small in the
# first groups (array still ramping its p-state) and in the last (short
# tail after the final DMA lands).
GROUPS = [
    (12, 4, 4),
    (29, 11, 3),
    (33, 11, 3),
    (33, 11, 3),
    (33, 11, 3),
    (25, 9, 2),
    (16, 6, 2),
    (8, 0, 0),
]
assert sum(p + a + d for p, a, d in GROUPS) == SLICES
NG = len(GROUPS)
MAXSZ = max(p + a + d for p, a, d in GROUPS)
NGRP = NG - 3  # groups 0..NGRP-1 feed psA; the rest feed psB
N_WARM = 27    # dummy PE matmuls to ramp the array's p-state before work

# gx is scaled by -2 on host, so the cross-term prods add directly into the
# same sign as the squares and every partial column is a plain addend.
# outcols: [G diag | merged G2 diag + prodB | ACT (NG) | DVE sq (NG) | prodA]
NC = 2 * NG + 3


def _build_nc():
    nc = bacc.Bacc()
    f32 = mybir.dt.float32
    bf16 = mybir.dt.bfloat16
    fp8 = mybir.dt.float8e4

    pred8 = nc.declare_dram_parameter("pred8", [H, SLICES, W], fp8, isOutput=False)
    gy8 = nc.declare_dram_parameter("gy8", [H, SLICES], fp8, isOutput=False)
    gx = nc.declare_dram_parameter("gx", [W, SLICES], bf16, isOutput=False)
    ident = nc.declare_dram_parameter("ident", [128, 128], bf16, isOutput=False)
    partials = nc.declare_dram_parameter("partials", [128, NC], f32, isOutput=True)

    n_main = sum(sum(g) for g in GROUPS[:NGRP])
    n_last = SLICES - n_main

    with tile.TileContext(nc) as tc:
        with (
            tc.tile_pool(name="consts", bufs=1) as consts,
            tc.tile_pool(name="l8", bufs=6) as l8pool,
            tc.tile_pool(name="work", bufs=1) as work,
            tc.tile_pool(name="gpsum", bufs=1, space="PSUM") as gpsum,
            tc.tile_pool(name="mpsum", bufs=1, space="PSUM") as mpsum,
            tc.tile_pool(name="outs", bufs=1) as outs,
        ):
            # warm-up ACT so the Square table-set load overlaps the DMA ramp
            warm = consts.tile([128, 1], f32)
            nc.vector.memset(warm[:], 0.0)
            wsq = consts.tile([128, 1], f32)
            nc.scalar.activation(
                out=wsq[:], in_=warm[:], func=mybir.ActivationFunctionType.Square
            )
            # warm up the PE array: dummy matmuls ramp its p-state to full
            # clock while the first pred chunk is still in flight
            wmat = consts.tile([128, 128], fp8)
            nc.vector.memset(wmat[:], 0.0)
            Gw = gpsum.tile([128, 128], f32, tag="Gw")
            for i in range(N_WARM):
                nc.tensor.matmul(
                    Gw[:], wmat[:], wmat[:], start=True, stop=True
                )

            gy8_t = consts.tile([H, SLICES], fp8)
            gx_t = consts.tile([W, SLICES], bf16)
            ident_t = consts.tile([128, 128], bf16)
            actout = consts.tile([128, max(a for _, a, _ in GROUPS) * W], bf16)
            sq16 = consts.tile([128, max(d for _, _, d in GROUPS) * W], bf16)
            outcols = outs.tile([128, NC], f32)

            psA = mpsum.tile([128, n_main], f32, tag="psA")
            psB = mpsum.tile([128, n_last], f32, tag="psB")

            # global Gram accumulator for groups 0..NG-3 (diagonal read out
            # right after the last of those chains, off the tail); the last
            # two groups' PE shares go to a small separate accumulator so
            # the tail chain is short
            G = gpsum.tile([128, 128], f32, tag="G")
            G2 = gpsum.tile([128, 128], f32, tag="G2")
            n_pe_main = sum(p for p, _, _ in GROUPS[:-2])
            n_pe_tail = sum(p for p, _, _ in GROUPS[-2:])
            pe_done = 0
            pe2_done = 0

            g0 = 0
            mm0 = None
            for r, (pk, ak, dk) in enumerate(GROUPS):
                sz = pk + ak + dk
                t8 = l8pool.tile([128, MAXSZ * W], fp8, tag="l8")
                nc.sync.dma_start(
                    out=t8[:, : sz * W],
                    in_=pred8[:, g0 : g0 + sz, :].rearrange("h s w -> h (s w)"),
                )
                if r == 1:
                    # small const loads slot in after the first two pred
                    # DMAs; their consumers (PE cross-term matmuls, grouped
                    # prods, per-group diag extraction) all have slack
                    nc.sync.dma_start(out=gy8_t[:], in_=gy8[:, :])
                    nc.sync.dma_start(out=gx_t[:], in_=gx[:, :])
                    nc.sync.dma_start(out=ident_t[:], in_=ident[:, :])

                # PE: Gram accumulation over its share -> trace = sum sq
                if r < NG - 2:
                    for i in range(pk):
                        nc.tensor.matmul(
                            G[:],
                            t8[:, i * W : (i + 1) * W],
                            t8[:, i * W : (i + 1) * W],
                            start=(pe_done + i == 0),
                            stop=(pe_done + i == n_pe_main - 1),
                        )
                    pe_done += pk
                else:
                    for i in range(pk):
                        nc.tensor.matmul(
                            G2[:],
                            t8[:, i * W : (i + 1) * W],
                            t8[:, i * W : (i + 1) * W],
                            start=(pe2_done + i == 0),
                            stop=(pe2_done + i == n_pe_tail - 1),
                        )
                    pe2_done += pk

                # ACT share: activation Square with accumulator
                if ak:
                    nc.scalar.activation(
                        out=actout[:, : ak * W],
                        in_=t8[:, pk * W : (pk + ak) * W],
                        func=mybir.ActivationFunctionType.Square,
                        accum_out=outcols[:, 2 + r : 3 + r],
                    )
                else:
                    nc.vector.memset(outcols[:, 2 + r : 3 + r], 0.0)

                # DVE share: square (fp8 -> bf16), halving tree, reduce
                if dk:
                    n = dk * W
                    o = (pk + ak) * W
                    nc.vector.tensor_mul(
                        sq16[:, :n], t8[:, o : o + n], t8[:, o : o + n]
                    )
                    while n > 256:
                        h = n // 2
                        nc.vector.tensor_add(
                            sq16[:, :h], sq16[:, :h], sq16[:, h:n]
                        )
                        n = h
                    nc.vector.reduce_sum(
                        outcols[:, 2 + NG + r : 3 + NG + r], sq16[:, :n],
                        axis=mybir.AxisListType.X,
                    )
                else:
                    nc.vector.memset(outcols[:, 2 + NG + r : 3 + NG + r], 0.0)

                # cross term: m'_s = pred_s^T @ gy_s per slice -> psum column
                ps, col0 = (psA, g0) if r < NGRP else (psB, g0 - n_main)

                def emit_matmuls(ps=ps, col0=col0, t8=t8, sz=sz, g0=g0):
                    for i in range(sz):
                        nc.tensor.matmul(
                            ps[:, col0 + i : col0 + i + 1],
                            t8[:, i * W : (i + 1) * W],
                            gy8_t[:, g0 + i : g0 + i + 1],
                            start=True,
                            stop=True,
                        )

                if r == 0:
                    # gy is only loaded during group 1; defer group 0's
                    # cross-term matmuls past it in program order
                    mm0 = emit_matmuls
                else:
                    if r == 1:
                        mm0()
                    emit_matmuls()

                if r == NGRP:
                    # grouped prod over the main groups' m' columns; its
                    # matmul wait resolved long ago, no queue blocking
                    prodA = work.tile([128, n_main], f32, tag="prodA")
                    nc.vector.tensor_mul(prodA[:], psA[:], gx_t[:, :n_main])
                    nc.vector.reduce_sum(
                        outcols[:, 2 * NG + 2 : 2 * NG + 3], prodA[:],
                        axis=mybir.AxisListType.X,
                    )

                if r == NG - 2:
                    # global Gram diagonal: runs as soon as the main G
                    # chain stops, while the last group still streams
                    dbuf = work.tile([128, 128], f32, tag="dbuf")
                    nc.vector.tensor_mul(dbuf[:], G[:], ident_t[:])
                    nc.vector.reduce_sum(
                        outcols[:, 0:1], dbuf[:], axis=mybir.AxisListType.X
                    )

                g0 += sz

            # tail: G2 diagonal and last groups' prod share one buffer and
            # one reduce (gx carries the -2, so both are plain s1 addends)
            tbuf = work.tile([128, 128 + n_last], f32, tag="tbuf")
            nc.vector.tensor_mul(tbuf[:, :128], G2[:], ident_t[:])
            nc.vector.tensor_mul(tbuf[:, 128:], psB[:], gx_t[:, n_main:])
            nc.vector.reduce_sum(
                outcols[:, 1:2], tbuf[:], axis=mybir.AxisListType.X
            )

            nc.sync.dma_start(out=partials[:, :], in_=outcols[:])

    nc.finalize()
    return nc


def _gaussians(proj_mats_batch, joints_3d_gt_batch):
    """1D gaussians gy [B,V,J,H], gx [B,V,J,W] in float32 (reference math)."""
    joints = joints_3d_gt_batch.astype(np.float32)
    ones = np.ones(joints.shape[:-1] + (1,), dtype=np.float32)
    joints_h = np.concatenate([joints, ones], axis=-1)  # [B, J, 4]
    proj = np.einsum(
        "bvcd,bjd->bvjc", proj_mats_batch.astype(np.float32), joints_h
    ).astype(np.float32)  # [B, V, J, 3]
    joints_2d = proj[..., :2] / proj[..., 2:3]  # (x, y)
    xs = np.arange(W, dtype=np.float32)
    ys = np.arange(H, dtype=np.float32)
    dx2 = (xs - joints_2d[..., 0, None]) ** 2  # [B,V,J,W]
    dy2 = (ys - joints_2d[..., 1, None]) ** 2  # [B,V,J,H]
    gx = np.exp(-0.5 * dx2).astype(np.float32)
    gy = np.exp(-0.5 * dy2).astype(np.float32)
    return gy, gx


def kernel(heatmaps_pred, proj_mats_batch, joints_3d_gt_batch, joints_3d_valid_batch,
           _profile=None):
    heatmaps_pred = np.asarray(heatmaps_pred, dtype=np.float32)
    gy, gx = _gaussians(np.asarray(proj_mats_batch), np.asarray(joints_3d_gt_batch))

    # s3 = sum over slices of (sum_h gy^2) * (sum_w gx^2), exact in f64
    s3 = float(
        ((gy.astype(np.float64) ** 2).sum(-1) * (gx.astype(np.float64) ** 2).sum(-1)).sum()
    )

    if "nc" not in _CACHE:
        _CACHE["nc"] = _build_nc()
    nc = _CACHE["nc"]

    ident = np.eye(128, dtype=ml_dtypes.bfloat16)
    in_maps = []
    for c in range(N_CORES):
        bsl = slice(B_LOC * c, B_LOC * (c + 1))
        # slice order: (b_local, v, j) -> s ; pred h-major [H, SLICES, W]
        pred8 = np.ascontiguousarray(
            heatmaps_pred[bsl].reshape(SLICES, H, W).transpose(1, 0, 2)
            .astype(ml_dtypes.float8_e4m3)
        )
        gy8 = np.ascontiguousarray(
            gy[bsl].reshape(SLICES, H).T.astype(ml_dtypes.float8_e4m3)
        )
        gx_c = np.ascontiguousarray(
            (-2.0 * gx[bsl].reshape(SLICES, W).T).astype(ml_dtypes.bfloat16)
        )
        in_maps.append({"pred8": pred8, "gy8": gy8, "gx": gx_c, "ident": ident})

    res = run_bass_kernel_spmd(nc, in_maps, core_ids=list(range(N_CORES)))
    if _profile is not None:
        _profile["result"] = res
        _profile["in_maps"] = in_maps

    # every partials column is a plain addend: squares partials plus the
    # cross-term prods (gx already carries the -2)
    total = s3
    for c in range(N_CORES):
        total += res.results[c]["partials"].astype(np.float64).sum()
    return np.float32(total / (B * V * J * H * W))


# revision 52
# speedup vs baseline: 1.0435x; 1.0046x over previous
"""HeatmapMSELoss Trainium2 kernel (fp8 stream + PE Gram-matmul squares).

Computes mean((heatmaps_pred - heatmaps_gt)^2) where heatmaps_gt is an
isotropic 2D gaussian (sigma=1, peak 1) rendered at the projection of each
3D joint into each view.

Key identity: the gaussian separates, gt[h,w] = gy[h] * gx[w], so

  sum_hw (pred - gt)^2 = sum_hw pred^2 - 2 * gy^T (pred @ gx) + (sum gy^2)(sum gx^2)

The gt tensor is never materialized. pred is pre-transposed on host to
h-major [H, S, W] and cast to fp8e4m3 (biases the final scalar by ~7e-4
relative, harness tolerance 2e-2), quartering HBM traffic vs f32 while
every DMA descriptor stays >= 512 contiguous bytes.

sum(pred^2) rides the (otherwise idle) PE systolic array: per group of
slices, PSUM-accumulated Gram matmuls G = sum_s pred_s^T pred_s, whose
trace is sum of squares. The diagonal is extracted once per group by one
DVE multiply against a preloaded identity mask and one reduce. The ACT
engine squares a minority share directly (activation Square + accum),
sized larger in the early groups while the PE array is still in its slow
p-state; DVE squares a few head slices it would otherwise idle through.
This makes the kernel memory-bound on the fp8 stream.

The cross term uses one per-slice matmul (pred_s^T @ gy_s) into a
persistent PSUM tile; the multiply by gx and the reduction run as two
grouped DVE ops.

Sharding: data-parallel over batch, 4 batches per core across 8 cores;
the tiny per-group column partials are combined on host in float64.
"""

import numpy as np
import ml_dtypes

import concourse.bacc as bacc
import concourse.bass as bass
import concourse.tile as tile
from concourse import mybir
from concourse.bass_utils import run_bass_kernel_spmd

B, V, J, H, W = 32, 4, 17, 128, 128
N_CORES = 8
B_LOC = B // N_CORES          # 4 batches per core
SLICES = B_LOC * V * J        # 272 slices per core

_CACHE = {}

# Per group: (pe_slices, act_slices, dve_slices). PE share is small in the
# first groups (array still ramping its p-state) and in the last (short
# tail after the final DMA lands).
GROUPS = [
    (12, 4, 4),
    (29, 11, 3),
    (33, 11, 3),
    (33, 11, 3),
    (33, 11, 3),
    (25, 9, 2),
    (16, 6, 2),
    (8, 0, 0),
]
assert sum(p + a + d for p, a, d in GROUPS) == SLICES
NG = len(GROUPS)
MAXSZ = max(p + a + d for p, a, d in GROUPS)
NGRP = NG - 3  # groups 0..NGRP-1 feed psA; the rest feed psB
N_WARM = 27    # dummy PE matmuls to ramp the array's p-state before work

# gx is scaled by -2 on host, so the cross-term prods add directly into the
# same sign as the squares and every partial column is a plain addend.
# outcols: [G diag | G2 diag | ACT (NG) | DVE sq (NG) | prodA | prodB]
NC = 2 * NG + 4


def _build_nc():
    nc = bacc.Bacc()
    f32 = mybir.dt.float32
    bf16 = mybir.dt.bfloat16
    fp8 = mybir.dt.float8e4

    pred8 = nc.declare_dram_parameter("pred8", [H, SLICES, W], fp8, isOutput=False)
    gy8 = nc.declare_dram_parameter("gy8", [H, SLICES], fp8, isOutput=False)
    gx = nc.declare_dram_parameter("gx", [W, SLICES], bf16, isOutput=False)
    ident = nc.declare_dram_parameter("ident", [128, 128], bf16, isOutput=False)
    partials = nc.declare_dram_parameter("partials", [128, NC], f32, isOutput=True)

    n_main = sum(sum(g) for g in GROUPS[:NGRP])
    n_last = SLICES - n_main

    with tile.TileContext(nc) as tc:
        with (
            tc.tile_pool(name="consts", bufs=1) as consts,
            tc.tile_pool(name="l8", bufs=6) as l8pool,
            tc.tile_pool(name="work", bufs=1) as work,
            tc.tile_pool(name="gpsum", bufs=1, space="PSUM") as gpsum,
            tc.tile_pool(name="mpsum", bufs=1, space="PSUM") as mpsum,
            tc.tile_pool(name="outs", bufs=1) as outs,
        ):
            # warm-up ACT so the Square table-set load overlaps the DMA ramp
            warm = consts.tile([128, 1], f32)
            nc.vector.memset(warm[:], 0.0)
            wsq = consts.tile([128, 1], f32)
            nc.scalar.activation(
                out=wsq[:], in_=warm[:], func=mybir.ActivationFunctionType.Square
            )
            # warm up the PE array: dummy matmuls ramp its p-state to full
            # clock while the first pred chunk is still in flight
            wmat = consts.tile([128, 128], fp8)
            nc.vector.memset(wmat[:], 0.0)
            Gw = gpsum.tile([128, 128], f32, tag="Gw")
            for i in range(N_WARM):
                nc.tensor.matmul(
                    Gw[:], wmat[:], wmat[:], start=True, stop=True
                )

            gy8_t = consts.tile([H, SLICES], fp8)
            gx_t = consts.tile([W, SLICES], bf16)
            ident_t = consts.tile([128, 128], bf16)
            actout = consts.tile([128, max(a for _, a, _ in GROUPS) * W], bf16)
            sq16 = consts.tile([128, max(d for _, _, d in GROUPS) * W], bf16)
            outcols = outs.tile([128, NC], f32)

            psA = mpsum.tile([128, n_main], f32, tag="psA")
            psB = mpsum.tile([128, n_last], f32, tag="psB")

            # global Gram accumulator for groups 0..NG-3 (diagonal read out
            # right after the last of those chains, off the tail); the last
            # two groups' PE shares go to a small separate accumulator so
            # the tail chain is short
            G = gpsum.tile([128, 128], f32, tag="G")
            G2 = gpsum.tile([128, 128], f32, tag="G2")
            n_pe_main = sum(p for p, _, _ in GROUPS[:-2])
            n_pe_tail = sum(p for p, _, _ in GROUPS[-2:])
            pe_done = 0
            pe2_done = 0

            g0 = 0
            mm0 = None
            for r, (pk, ak, dk) in enumerate(GROUPS):
                sz = pk + ak + dk
                t8 = l8pool.tile([128, MAXSZ * W], fp8, tag="l8")
                nc.sync.dma_start(
                    out=t8[:, : sz * W],
                    in_=pred8[:, g0 : g0 + sz, :].rearrange("h s w -> h (s w)"),
                )
                if r == 1:
                    # small const loads slot in after the first two pred
                    # DMAs; their consumers (PE cross-term matmuls, grouped
                    # prods, per-group diag extraction) all have slack
                    nc.sync.dma_start(out=gy8_t[:], in_=gy8[:, :])
                    nc.sync.dma_start(out=gx_t[:], in_=gx[:, :])
                    nc.sync.dma_start(out=ident_t[:], in_=ident[:, :])

                # PE: Gram accumulation over its share -> trace = sum sq
                if r < NG - 2:
                    for i in range(pk):
                        nc.tensor.matmul(
                            G[:],
                            t8[:, i * W : (i + 1) * W],
                            t8[:, i * W : (i + 1) * W],
                            start=(pe_done + i == 0),
                            stop=(pe_done + i == n_pe_main - 1),
                        )
                    pe_done += pk
                else:
                    for i in range(pk):
                        nc.tensor.matmul(
                            G2[:],
                            t8[:, i * W : (i + 1) * W],
                            t8[:, i * W : (i + 1) * W],
                            start=(pe2_done + i == 0),
                            stop=(pe2_done + i == n_pe_tail - 1),
                        )
                    pe2_done += pk

                # ACT share: activation Square with accumulator
                if ak:
                    nc.scalar.activation(
                        out=actout[:, : ak * W],
                        in_=t8[:, pk * W : (pk + ak) * W],
                        func=mybir.ActivationFunctionType.Square,
                        accum_out=outcols[:, 2 + r : 3 + r],
                    )
                else:
                    nc.vector.memset(outcols[:, 2 + r : 3 + r], 0.0)

                # DVE share: square (fp8 -> bf16), halving tree, reduce
                if dk:
                    n = dk * W
                    o = (pk + ak) * W
                    nc.vector.tensor_mul(
                        sq16[:, :n], t8[:, o : o + n], t8[:, o : o + n]
                    )
                    while n > 256:
                        h = n // 2
                        nc.vector.tensor_add(
                            sq16[:, :h], sq16[:, :h], sq16[:, h:n]
                        )
                        n = h
                    nc.vector.reduce_sum(
                        outcols[:, 2 + NG + r : 3 + NG + r], sq16[:, :n],
                        axis=mybir.AxisListType.X,
                    )
                else:
                    nc.vector.memset(outcols[:, 2 + NG + r : 3 + NG + r], 0.0)

                # cross term: m'_s = pred_s^T @ gy_s per slice -> psum column
                ps, col0 = (psA, g0) if r < NGRP else (psB, g0 - n_main)

                def emit_matmuls(ps=ps, col0=col0, t8=t8, sz=sz, g0=g0):
                    for i in range(sz):
                        nc.tensor.matmul(
                            ps[:, col0 + i : col0 + i + 1],
                            t8[:, i * W : (i + 1) * W],
                            gy8_t[:, g0 + i : g0 + i + 1],
                            start=True,
                            stop=True,
                        )

                if r == 0:
                    # gy is only loaded during group 1; defer group 0's
                    # cross-term matmuls past it in program order
                    mm0 = emit_matmuls
                else:
                    if r == 1:
                        mm0()
                    emit_matmuls()

                if r == NGRP:
                    # grouped prod over the main groups' m' columns; its
                    # matmul wait resolved long ago, no queue blocking
                    prodA = work.tile([128, n_main], f32, tag="prodA")
                    nc.vector.tensor_mul(prodA[:], psA[:], gx_t[:, :n_main])
                    nc.vector.reduce_sum(
                        outcols[:, 2 * NG + 2 : 2 * NG + 3], prodA[:],
                        axis=mybir.AxisListType.X,
                    )

                if r == NG - 2:
                    # global Gram diagonal: runs as soon as the main G
                    # chain stops, while the last group still streams
                    dbuf = work.tile([128, 128], f32, tag="dbuf")
                    nc.vector.tensor_mul(dbuf[:], G[:], ident_t[:])
                    nc.vector.reduce_sum(
                        outcols[:, 0:1], dbuf[:], axis=mybir.AxisListType.X
                    )

                g0 += sz

            # tail: small G2 diagonal + last groups' prod (gx carries the
            # -2, so both are plain s1 addends)
            dbuf2 = work.tile([128, 128], f32, tag="dbuf2")
            nc.vector.tensor_mul(dbuf2[:], G2[:], ident_t[:])
            nc.vector.reduce_sum(
                outcols[:, 1:2], dbuf2[:], axis=mybir.AxisListType.X
            )

            prodB = work.tile([128, n_last], f32, tag="prodB")
            nc.vector.tensor_mul(prodB[:], psB[:], gx_t[:, n_main:])
            nc.vector.reduce_sum(
                outcols[:, 2 * NG + 3 : 2 * NG + 4], prodB[:],
                axis=mybir.AxisListType.X,
            )

            nc.sync.dma_start(out=partials[:, :], in_=outcols[:])

    nc.finalize()
    return nc


def _gaussians(proj_mats_batch, joints_3d_gt_batch):
    """1D gaussians gy [B,V,J,H], gx [B,V,J,W] in float32 (reference math)."""
    joints = joints_3d_gt_batch.astype(np.float32)
    ones = np.ones(joints.shape[:-1] + (1,), dtype=np.float32)
    joints_h = np.concatenate([joints, ones], axis=-1)  # [B, J, 4]
    proj = np.einsum(
        "bvcd,bjd->bvjc", proj_mats_batch.astype(np.float32), joints_h
    ).astype(np.float32)  # [B, V, J, 3]
    joints_2d = proj[..., :2] / proj[..., 2:3]  # (x, y)
    xs = np.arange(W, dtype=np.float32)
    ys = np.arange(H, dtype=np.float32)
    dx2 = (xs - joints_2d[..., 0, None]) ** 2  # [B,V,J,W]
    dy2 = (ys - joints_2d[..., 1, None]) ** 2  # [B,V,J,H]
    gx = np.exp(-0.5 * dx2).astype(np.float32)
    gy = np.exp(-0.5 * dy2).astype(np.float32)
    return gy, gx


def kernel(heatmaps_pred, proj_mats_batch, joints_3d_gt_batch, joints_3d_valid_batch,
           _profile=None):
    heatmaps_pred = np.asarray(heatmaps_pred, dtype=np.float32)
    gy, gx = _gaussians(np.asarray(proj_mats_batch), np.asarray(joints_3d_gt_batch))

    # s3 = sum over slices of (sum_h gy^2) * (sum_w gx^2), exact in f64
    s3 = float(
        ((gy.astype(np.float64) ** 2).sum(-1) * (gx.astype(np.float64) ** 2).sum(-1)).sum()
    )

    if "nc" not in _CACHE:
        _CACHE["nc"] = _build_nc()
    nc = _CACHE["nc"]

    ident = np.eye(128, dtype=ml_dtypes.bfloat16)
    in_maps = []
    for c in range(N_CORES):
        bsl = slice(B_LOC * c, B_LOC * (c + 1))
        # slice order: (b_local, v, j) -> s ; pred h-major [H, SLICES, W]
        pred8 = np.ascontiguousarray(
            heatmaps_pred[bsl].reshape(SLICES, H, W).transpose(1, 0, 2)
            .astype(ml_dtypes.float8_e4m3)
        )
        gy8 = np.ascontiguousarray(
            gy[bsl].reshape(SLICES, H).T.astype(ml_dtypes.float8_e4m3)
        )
        gx_c = np.ascontiguousarray(
            (-2.0 * gx[bsl].reshape(SLICES, W).T).astype(ml_dtypes.bfloat16)
        )
        in_maps.append({"pred8": pred8, "gy8": gy8, "gx": gx_c, "ident": ident})

    res = run_bass_kernel_spmd(nc, in_maps, core_ids=list(range(N_CORES)))
    if _profile is not None:
        _profile["result"] = res
        _profile["in_maps"] = in_maps

    # every partials column is a plain addend: squares partials plus the
    # cross-term prods (gx already carries the -2)
    total = s3
    for c in range(N_CORES):
        total += res.results[c]["partials"].astype(np.float64).sum()
    return np.float32(total / (B * V * J * H * W))
